# revision 46
# baseline (speedup 1.0000x reference)
"""Trainium2 Bass kernel for CrossAttention (B=8, N=M=2048, C=1024), fp32 in/out.

Sharding: data-parallel — one batch element per NeuronCore (8 cores).

On top of the bf16 version's optimizations (mask packing, SBUF residency,
Q-first phase order, dependency-clean DMA issue order), the five large
matmuls (q/k/v, s = q k^T, and the output projection) run as fp8-e4m3
DoubleRow matmuls with hi/lo error compensation:

    A @ B  ~=  A_hi @ B_hi + A_hi @ B_lo + A_lo @ B_hi,

where X_hi = fp8(X) and X_lo = fp8(X - X_hi).  DoubleRow processes a
256-deep contraction at 0.5 cycles/row, so three compensated passes cost
0.75x of one bf16 pass per 128-contraction — a 25% tensor-engine saving —
while the residual term makes the operand error ~2x SMALLER than bf16.
All accumulation stays fp32 in PSUM.  Scales: weights are pre-scaled by
32 on the host (w std ~1/32 -> fp8 normal range); activations flow as
32q/32k/32v, the exp scale absorbs 1/1024, o is stored as 32*o, and the
projection descales by 1/1024 on the scalar engine before the bias add.
p (post-exp, unbounded above) and the p@v matmul stay bf16 — p cannot
be ranged into fp8 without max-subtraction.

Per-core computation (batch b):
  qT[d, n] = (x[b] @ wq^T)^T
  kT[d, m] = (support_perm[b] @ wk^T)^T                (all m)
  v[m, d]  = (support_perm[b] @ wv^T) * mask_perm[m]   (m < mv only)
  p = exp(SCALE * q k^T)  (no max subtraction: logits ~ N(0, 8), safe fp32)
  o[n, d] = (p[:, :mv] @ v) / rowsum_all_m(p)
  out[2t+i, d'] = sum_c o[1024 i + c, t] * proj_w[d', c] + proj_b[d']
"""

import sys

sys.path.insert(0, "/opt/trn_rl_repo")

import numpy as np
import ml_dtypes

import concourse.bass as bass
import concourse.tile as tile
from concourse import bacc, mybir
from concourse.bass_utils import run_bass_kernel_spmd
from concourse.masks import make_identity

F32 = mybir.dt.float32
BF16 = mybir.dt.bfloat16
E4 = mybir.dt.float8e4
AF = mybir.ActivationFunctionType
DR = mybir.MatmulPerfMode.DoubleRow
NPBF = ml_dtypes.bfloat16
NPE4 = ml_dtypes.float8_e4m3

B, N, M, C = 8, 2048, 2048, 1024
CT = C // 128          # 8 c-tiles (contraction / channel partition tiles)
MT = M // 128          # 16 m-tiles
SCALE = (C // 8) ** -0.5
WS = 32.0              # host weight upscale into the fp8 normal range
NCHUNK = 256           # q rows computed per chunk in the q phase
MS = 512               # m-chunk for kv build and the s matmul free dim

_CACHE = {}


def _build_program(mt_u):
    nc = bacc.Bacc(
        "TRN2",
        target_bir_lowering=False,
        debug=False,
        enable_asserts=False,
        num_devices=8,
    )

    NW = N // NCHUNK
    MW = M // MS
    xhl = nc.dram_tensor("xhl", [128, NW, 2, CT, NCHUNK], E4, kind="ExternalInput")
    shl = nc.dram_tensor("shl", [128, MW, 2, CT, MS], E4, kind="ExternalInput")
    wqh = nc.dram_tensor("wqh", [128, CT, CT, 128], E4, kind="ExternalInput")
    wql = nc.dram_tensor("wql", [128, CT, CT, 128], E4, kind="ExternalInput")
    wvh = nc.dram_tensor("wvh", [128, 2, CT, 512], E4, kind="ExternalInput")
    wvl = nc.dram_tensor("wvl", [128, 2, CT, 512], E4, kind="ExternalInput")
    pwh = nc.dram_tensor("pwh", [128, 2, CT, 512], E4, kind="ExternalInput")
    pwl = nc.dram_tensor("pwl", [128, 2, CT, 512], E4, kind="ExternalInput")
    maskf = nc.dram_tensor("maskf", [128, mt_u], F32, kind="ExternalInput")
    biasb = nc.dram_tensor("biasb", [128, C], BF16, kind="ExternalInput")
    out = nc.dram_tensor("out", [N, C], F32, kind="ExternalOutput")

    with tile.TileContext(nc, pool_alloc_mode="queue") as tc:
        _trace_kernel(tc, mt_u, xhl, shl, wqh, wql,
                      wvh, wvl, pwh, pwl, maskf, biasb, out)
    nc.compile()
    return nc


def _trace_kernel(tc, mt_u, xhl, shl, wqh, wql,
                  wvh, wvl, pwh, pwl, maskf, biasb, out):
    nc = tc.nc
    mv = mt_u * 128
    NW = N // NCHUNK
    MW = M // MS

    from contextlib import ExitStack

    with ExitStack() as ctx:
        persist = ctx.enter_context(tc.tile_pool(name="persist", bufs=1))

        ident0 = persist.tile([128, 128], F32, tag="ident0")
        make_identity(nc, ident0[:])
        ident = persist.tile([128, 128], BF16, tag="ident")
        nc.scalar.copy(ident[:], ident0[:])

        # long-lived tiles (LIFO pool stack: these close only at the end)
        qtp = ctx.enter_context(tc.tile_pool(name="qtp", bufs=1))
        qt_h = qtp.tile([128, CT, N], E4, tag="qt_h")
        qt_l = qtp.tile([128, CT, N], E4, tag="qt_l")
        vp = ctx.enter_context(tc.tile_pool(name="vp", bufs=1))
        v = vp.tile([128, mt_u, C], BF16, tag="v")
        # support^T (hi/lo) doubles as the s-matmul rhs (k is folded into
        # the q weights on the host: s = (x G) sup^T with G = wq^T wk), so
        # it stays resident through attention
        stp = ctx.enter_context(tc.tile_pool(name="stp", bufs=1))
        st = stp.tile([128, MW, 2, CT, MS], E4, tag="st")
        o_p = ctx.enter_context(tc.tile_pool(name="op", bufs=1))
        o_h = o_p.tile([128, N // 128, C], E4, tag="o_h")
        o_l = o_p.tile([128, N // 128, C], E4, tag="o_l")
        pwp = ctx.enter_context(tc.tile_pool(name="pwp", bufs=1, side="right"))
        pw_h = pwp.tile([128, 2, CT, 512], E4, tag="pw_h")
        pw_l = pwp.tile([128, 2, CT, 512], E4, tag="pw_l")
        wvp = ctx.enter_context(tc.tile_pool(name="wvp", bufs=1, side="right"))
        wv_h = wvp.tile([128, 2, CT, 512], E4, tag="wv_h")
        wv_l = wvp.tile([128, 2, CT, 512], E4, tag="wv_l")

        maskt = persist.tile([128, mt_u], F32, tag="maskt")
        bias = persist.tile([128, C], BF16, tag="bias")

        # one psum pool shared by the Q and V phases: no bank-reuse
        # barrier at the phase seam
        qkv_ps = ctx.enter_context(ExitStack())
        qps = qkv_ps.enter_context(tc.tile_pool(name="qps", bufs=4, space="PSUM"))

        # ---------------- phase Q: qT[d, n] = (x @ wq^T)^T -----------------
        # wq is dt-major [p, dt, ct, dd]; hi/lo DoubleRow over ct-pairs
        with (
            tc.tile_pool(name="wqp", bufs=1, side="right") as wqp,
            tc.tile_pool(name="xq", bufs=N // NCHUNK) as xqp,
        ):
            wq_h = wqp.tile([128, CT, CT, 128], E4, tag="wq_h")
            wq_l = wqp.tile([128, CT, CT, 128], E4, tag="wq_l")

            def load_xq(i):
                t = xqp.tile([128, 2, CT, NCHUNK], E4, tag="xq")
                nc.sync.dma_start(t[:, :, :, :], xhl[:, i, :, :, :])
                return t

            # ramp-up: x chunk 0 + wq pieces paced with the Q loop, then the
            # remaining x chunks and the K/V working set — all dependency-
            # free, so nothing head-of-line-blocks the DMA issue queue.
            xq0 = xqp.tile([128, 2, CT, NCHUNK], E4, tag="xq")
            nc.sync.dma_start(xq0[:, 0, :, :], xhl[:, 0, 0, :, :])
            nc.sync.dma_start(wq_h[:, 0, :, :], wqh[:, 0, :, :])
            nc.sync.dma_start(wq_l[:, 0, :, :], wql[:, 0, :, :])
            nc.sync.dma_start(xq0[:, 1, :, :], xhl[:, 0, 1, :, :])
            xqs = [xq0]
            for dt in range(1, CT):
                nc.sync.dma_start(wq_h[:, dt, :, :], wqh[:, dt, :, :])
                nc.sync.dma_start(wq_l[:, dt, :, :], wql[:, dt, :, :])
                if dt in (3, 6):  # pace x chunks 1-2 between weight pieces
                    xqs.append(load_xq(len(xqs)))
            for i in range(len(xqs), NW):
                xqs.append(load_xq(i))
            nc.sync.dma_start(maskt[:], maskf[:])
            nc.sync.dma_start(bias[:], biasb[:])
            for mc in range(MW):
                nc.sync.dma_start(st[:, mc, :, :, :], shl[:, mc, :, :, :])
            for dc in range(2):
                nc.sync.dma_start(wv_h[:, dc, :, :], wvh[:, dc, :, :])
                nc.sync.dma_start(wv_l[:, dc, :, :], wvl[:, dc, :, :])
            for dc in range(2):
                nc.sync.dma_start(pw_h[:, dc, :, :], pwh[:, dc, :, :])
                nc.sync.dma_start(pw_l[:, dc, :, :], pwl[:, dc, :, :])

            for nch in range(NW):
                xq = xqs[nch]
                for dt in range(CT):
                    ps = qps.tile([128, NCHUNK], F32, tag="qps")
                    series = [(wq_h, 0), (wq_h, 1), (wq_l, 0)]
                    for si, (w_, xi) in enumerate(series):
                        for j in range(CT // 2):
                            nc.tensor.matmul(
                                ps[:],
                                lhsT=w_[:, dt, 2 * j:2 * j + 2, :],
                                rhs=xq[:, xi, 2 * j:2 * j + 2, :],
                                start=(si == 0 and j == 0),
                                stop=(si == 2 and j == CT // 2 - 1),
                                perf_mode=DR,
                            )
                    win = slice(nch * NCHUNK, (nch + 1) * NCHUNK)
                    nc.scalar.copy(qt_h[:, dt, win], ps[:])
                    nc.vector.tensor_sub(qt_l[:, dt, win], ps[:], qt_h[:, dt, win])

        # ---------------- phase V: v[m, d] = mask * (support @ wv^T) -------
        for mt in range(mt_u):
            mc, jj = divmod(mt, MS // 128)
            for dc in range(C // 512):
                ps = qps.tile([128, 512], F32, tag="qps")
                series = [(0, wv_h), (0, wv_l), (1, wv_h)]
                for si, (si2, w_) in enumerate(series):
                    for j in range(CT // 2):
                        nc.tensor.matmul(
                            ps[:],
                            lhsT=st[:, mc, si2, 2 * j:2 * j + 2, jj * 128:(jj + 1) * 128],
                            rhs=w_[:, dc, 2 * j:2 * j + 2, :],
                            start=(si == 0 and j == 0),
                            stop=(si == 2 and j == CT // 2 - 1),
                            perf_mode=DR,
                        )
                nc.vector.tensor_scalar_mul(
                    v[:, mt, dc * 512:(dc + 1) * 512],
                    ps[:],
                    maskt[:, mt:mt + 1],
                )

        qkv_ps.close()

        # ---------------- attention: s / exp / transpose / p@v -------------
        with (
            tc.tile_pool(name="sps", bufs=2, space="PSUM") as sps,
            tc.tile_pool(name="ptps", bufs=2, space="PSUM") as ptps,
            tc.tile_pool(name="ops", bufs=2, space="PSUM") as ops,
            tc.tile_pool(name="psb", bufs=3) as psbp,
            tc.tile_pool(name="ptsb", bufs=2) as ptsbp,
            tc.tile_pool(name="obf", bufs=2) as obfp,
            tc.tile_pool(name="stat", bufs=4) as statp,
        ):
            def transpose_and_pv(g, p_sb, o_ps):
                pt_ps = ptps.tile([128, MS], BF16, tag="ptps")
                for j in range(MS // 128):
                    nc.tensor.transpose(
                        pt_ps[:, j * 128:(j + 1) * 128],
                        p_sb[:, j * 128:(j + 1) * 128],
                        ident[:],
                    )
                pt_sb = ptsbp.tile([128, MS], BF16, tag="ptsb")
                nc.vector.tensor_copy(pt_sb[:], pt_ps[:])
                for j in range(MS // 128):
                    mt = g * (MS // 128) + j
                    for dc in range(C // 512):
                        nc.tensor.matmul(
                            o_ps[:, dc * 512:(dc + 1) * 512],
                            lhsT=pt_sb[:, j * 128:(j + 1) * 128],
                            rhs=v[:, mt, dc * 512:(dc + 1) * 512],
                            start=(mt == 0),
                            stop=(mt == mt_u - 1),
                        )

            for ntile in range(N // 128):
                partials = statp.tile([128, 4], F32, tag="partials")
                o_ps = ops.tile([128, C], F32, tag="ops")
                for g in range(M // MS):
                    s_ps = sps.tile([128, MS], F32, tag="sps")
                    series = [(qt_h, 0), (qt_h, 1), (qt_l, 0)]
                    for si, (q_, si2) in enumerate(series):
                        for j in range(CT // 2):
                            nc.tensor.matmul(
                                s_ps[:],
                                lhsT=q_[:, 2 * j:2 * j + 2, ntile * 128:(ntile + 1) * 128],
                                rhs=st[:, g, si2, 2 * j:2 * j + 2, :],
                                start=(si == 0 and j == 0),
                                stop=(si == 2 and j == CT // 2 - 1),
                                perf_mode=DR,
                            )
                    p_sb = psbp.tile([128, MS], BF16, tag="psb")
                    nc.scalar.activation(
                        p_sb[:], s_ps[:], AF.Exp,
                        scale=float(SCALE / WS),
                        accum_out=partials[:, g:g + 1],
                    )
                    if g * MS < mv:
                        transpose_and_pv(g, p_sb, o_ps)
                denom = statp.tile([128, 1], F32, tag="denom")
                nc.vector.reduce_sum(
                    denom[:], partials[:], axis=mybir.AxisListType.X
                )
                recip = statp.tile([128, 1], F32, tag="recip")
                nc.vector.reciprocal(recip[:], denom[:])
                # o_ps = 32*(p@v_true); o_bf = 32*o_true -> fp8 hi/lo
                o_bf = obfp.tile([128, C], BF16, tag="obf")
                nc.vector.tensor_scalar_mul(o_bf[:], o_ps[:], recip[:])
                nc.scalar.copy(o_h[:, ntile, :], o_bf[:])
                nc.vector.tensor_sub(o_l[:, ntile, :], o_bf[:], o_h[:, ntile, :])

        # ---------------- projection with the swapaxes/reshape fold --------
        # out[2t+i, d'] = (sum_c (32 o)[1024 i + c, t] (32 pw)[d', c]) / 1024
        #                 + bias
        with (
            tc.tile_pool(name="fps", bufs=4, space="PSUM") as fps,
            tc.tile_pool(name="fmid", bufs=4) as fmidp,
            tc.tile_pool(name="fsb", bufs=6) as fsbp,
        ):
            out_v = out[:].rearrange("(t two) d -> two t d", two=2)

            def proj_group(i, dc, tt, c0, cw):
                # one [128, cw] psum group of output columns dc*512+c0 ..
                ps = fps.tile([128, cw], F32, tag="fps")
                series = [(o_h, pw_h), (o_h, pw_l), (o_l, pw_h)]
                for si, (o_, w_) in enumerate(series):
                    for j in range(CT // 2):
                        nc.tensor.matmul(
                            ps[:],
                            lhsT=o_[:, CT * i + 2 * j: CT * i + 2 * j + 2, tt * 128:(tt + 1) * 128],
                            rhs=w_[:, dc, 2 * j:2 * j + 2, c0:c0 + cw],
                            start=(si == 0 and j == 0),
                            stop=(si == 2 and j == CT // 2 - 1),
                            perf_mode=DR,
                        )
                f_mid = fmidp.tile([128, cw], F32, tag="fmid")
                nc.scalar.mul(f_mid[:], ps[:], float(1.0 / (WS * WS)))
                f_sb = fsbp.tile([128, cw], F32, tag="fsb")
                nc.vector.tensor_add(
                    f_sb[:], f_mid[:],
                    bias[:, dc * 512 + c0: dc * 512 + c0 + cw],
                )
                nc.sync.dma_start(
                    out_v[i, tt * 128:(tt + 1) * 128, dc * 512 + c0: dc * 512 + c0 + cw],
                    f_sb[:],
                )

            for i in range(2):
                for dc in range(C // 512):
                    for tt in range(CT):
                        if (i, dc, tt) != (1, C // 512 - 1, CT - 1):
                            proj_group(i, dc, tt, 0, 512)
                        else:
                            # split the final group so its drain chain
                            # pipelines instead of sitting in the tail
                            for h in range(2):
                                proj_group(i, dc, tt, h * 256, 256)


def _hilo(a):
    h = np.clip(a, -240.0, 240.0).astype(NPE4)
    l = (a - h.astype(np.float32)).astype(NPE4)
    return np.ascontiguousarray(h), np.ascontiguousarray(l)


def _prep_w_lhs(w):
    # lhsT weights (wk, wq): [p, dt, ct, dd], scaled by WS, fp8 hi/lo
    wt = (w.T * WS).reshape(CT, 128, CT, 128)   # [ct, p, dt, dd]
    return _hilo(wt.transpose(1, 2, 0, 3))


def _prep_w_rhs(w):
    # rhs weights (wv, pw): [p, dc, ct, dd], scaled by WS, fp8 hi/lo
    wt = (w.T * WS).reshape(CT, 128, C // 512, 512)  # [ct, p, dc, dd]
    return _hilo(wt.transpose(1, 2, 0, 3))


def _prep_act(a, win):
    # a [rows, C] -> a.T [C, rows] window-major [p, w, hi/lo, ct, win] fp8
    n = a.shape[0]
    at = a.T.reshape(CT, 128, n // win, win)    # [ct, p, w, win]
    h, l = _hilo(at.transpose(1, 2, 0, 3))      # [p, w, ct, win]
    return np.ascontiguousarray(np.stack([h, l], axis=2))


def _mask_perm(attn_mask):
    # permutation packing unmasked support rows first; tile count for packed v
    mask = np.asarray(attn_mask)
    perm = np.argsort(mask == 0, kind="stable")
    cnt = int((mask != 0).sum())
    mt_u = max(1, min(MT, -(-cnt // 128)))
    return perm, mt_u


def prep_in_maps(x, support, attn_mask, qkv_w, proj_w, proj_b):
    x = np.asarray(x, dtype=np.float32)
    support = np.asarray(support, dtype=np.float32)
    attn_mask = np.asarray(attn_mask)
    qkv_w = np.asarray(qkv_w, dtype=np.float32)
    proj_w = np.asarray(proj_w, dtype=np.float32)
    proj_b = np.asarray(proj_b, dtype=np.float32)

    perm, mt_u = _mask_perm(attn_mask)
    maskp = attn_mask[perm].astype(np.float32)

    # fold k's projection into q's: s = q k^T = x (wq^T wk) sup^T
    G = qkv_w[:C].T @ qkv_w[C:2 * C]
    wq_h, wq_l = _prep_w_lhs(G.T)
    wv_h, wv_l = _prep_w_rhs(qkv_w[2 * C:])
    pw_h, pw_l = _prep_w_rhs(proj_w)
    maskf = np.ascontiguousarray(
        maskp[:mt_u * 128].reshape(mt_u, 128).T
    )
    biasb = np.ascontiguousarray(np.broadcast_to(proj_b, (128, C)).astype(NPBF))

    in_maps = []
    for b in range(B):
        in_maps.append({
            "xhl": _prep_act(x[b], NCHUNK),
            "shl": _prep_act(support[b][perm], MS),
            "wqh": wq_h, "wql": wq_l,
            "wvh": wv_h, "wvl": wv_l,
            "pwh": pw_h, "pwl": pw_l,
            "maskf": maskf,
            "biasb": biasb,
        })
    return in_maps


def kernel(x, support, attn_mask, qkv_w, proj_w, proj_b):
    _, mt_u = _mask_perm(attn_mask)
    if ("nc", mt_u) not in _CACHE:
        _CACHE[("nc", mt_u)] = _build_program(mt_u)
        _CACHE["nc"] = _CACHE[("nc", mt_u)]
    nc = _CACHE[("nc", mt_u)]

    in_maps = prep_in_maps(x, support, attn_mask, qkv_w, proj_w, proj_b)
    res = run_bass_kernel_spmd(nc, in_maps, core_ids=list(range(B)))
    return np.stack([res.results[b]["out"] for b in range(B)], axis=0)


# revision 47
# speedup vs baseline: 1.0013x; 1.0013x over previous
"""Trainium2 Bass kernel for CrossAttention (B=8, N=M=2048, C=1024), fp32 in/out.

Sharding: data-parallel — one batch element per NeuronCore (8 cores).

On top of the bf16 version's optimizations (mask packing, SBUF residency,
Q-first phase order, dependency-clean DMA issue order), the five large
matmuls (q/k/v, s = q k^T, and the output projection) run as fp8-e4m3
DoubleRow matmuls with hi/lo error compensation:

    A @ B  ~=  A_hi @ B_hi + A_hi @ B_lo + A_lo @ B_hi,

where X_hi = fp8(X) and X_lo = fp8(X - X_hi).  DoubleRow processes a
256-deep contraction at 0.5 cycles/row, so three compensated passes cost
0.75x of one bf16 pass per 128-contraction — a 25% tensor-engine saving —
while the residual term makes the operand error ~2x SMALLER than bf16.
All accumulation stays fp32 in PSUM.  Scales: weights are pre-scaled by
32 on the host (w std ~1/32 -> fp8 normal range); activations flow as
32q/32k/32v, the exp scale absorbs 1/1024, o is stored as 32*o, and the
projection descales by 1/1024 on the scalar engine before the bias add.
p (post-exp, unbounded above) and the p@v matmul stay bf16 — p cannot
be ranged into fp8 without max-subtraction.

Per-core computation (batch b):
  qT[d, n] = (x[b] @ wq^T)^T
  kT[d, m] = (support_perm[b] @ wk^T)^T                (all m)
  v[m, d]  = (support_perm[b] @ wv^T) * mask_perm[m]   (m < mv only)
  p = exp(SCALE * q k^T)  (no max subtraction: logits ~ N(0, 8), safe fp32)
  o[n, d] = (p[:, :mv] @ v) / rowsum_all_m(p)
  out[2t+i, d'] = sum_c o[1024 i + c, t] * proj_w[d', c] + proj_b[d']
"""

import sys

sys.path.insert(0, "/opt/trn_rl_repo")

import numpy as np
import ml_dtypes

import concourse.bass as bass
import concourse.tile as tile
from concourse import bacc, mybir
from concourse.bass_utils import run_bass_kernel_spmd
from concourse.masks import make_identity

F32 = mybir.dt.float32
BF16 = mybir.dt.bfloat16
E4 = mybir.dt.float8e4
AF = mybir.ActivationFunctionType
DR = mybir.MatmulPerfMode.DoubleRow
NPBF = ml_dtypes.bfloat16
NPE4 = ml_dtypes.float8_e4m3

B, N, M, C = 8, 2048, 2048, 1024
CT = C // 128          # 8 c-tiles (contraction / channel partition tiles)
MT = M // 128          # 16 m-tiles
SCALE = (C // 8) ** -0.5
WS = 32.0              # host weight upscale into the fp8 normal range
NCHUNK = 256           # q rows computed per chunk in the q phase
MS = 512               # m-chunk for kv build and the s matmul free dim

_CACHE = {}


def _build_program(mt_u):
    nc = bacc.Bacc(
        "TRN2",
        target_bir_lowering=False,
        debug=False,
        enable_asserts=False,
        num_devices=8,
    )

    NW = N // NCHUNK
    MW = M // MS
    xhl = nc.dram_tensor("xhl", [128, NW, 2, CT, NCHUNK], E4, kind="ExternalInput")
    shl = nc.dram_tensor("shl", [128, MW, 2, CT, MS], E4, kind="ExternalInput")
    wqh = nc.dram_tensor("wqh", [128, CT, CT, 128], E4, kind="ExternalInput")
    wql = nc.dram_tensor("wql", [128, CT, CT, 128], E4, kind="ExternalInput")
    wvh = nc.dram_tensor("wvh", [128, 2, CT, 512], E4, kind="ExternalInput")
    wvl = nc.dram_tensor("wvl", [128, 2, CT, 512], E4, kind="ExternalInput")
    pwh = nc.dram_tensor("pwh", [128, 2, CT, 512], E4, kind="ExternalInput")
    pwl = nc.dram_tensor("pwl", [128, 2, CT, 512], E4, kind="ExternalInput")
    maskf = nc.dram_tensor("maskf", [128, mt_u], F32, kind="ExternalInput")
    biasb = nc.dram_tensor("biasb", [128, C], BF16, kind="ExternalInput")
    out = nc.dram_tensor("out", [N, C], F32, kind="ExternalOutput")

    with tile.TileContext(nc, pool_alloc_mode="queue") as tc:
        _trace_kernel(tc, mt_u, xhl, shl, wqh, wql,
                      wvh, wvl, pwh, pwl, maskf, biasb, out)
    nc.compile()
    return nc


def _trace_kernel(tc, mt_u, xhl, shl, wqh, wql,
                  wvh, wvl, pwh, pwl, maskf, biasb, out):
    nc = tc.nc
    mv = mt_u * 128
    NW = N // NCHUNK
    MW = M // MS

    from contextlib import ExitStack

    with ExitStack() as ctx:
        persist = ctx.enter_context(tc.tile_pool(name="persist", bufs=1))

        ident0 = persist.tile([128, 128], F32, tag="ident0")
        make_identity(nc, ident0[:])
        ident = persist.tile([128, 128], BF16, tag="ident")
        nc.scalar.copy(ident[:], ident0[:])

        # long-lived tiles (LIFO pool stack: these close only at the end)
        qtp = ctx.enter_context(tc.tile_pool(name="qtp", bufs=1))
        qt_h = qtp.tile([128, CT, N], E4, tag="qt_h")
        qt_l = qtp.tile([128, CT, N], E4, tag="qt_l")
        vp = ctx.enter_context(tc.tile_pool(name="vp", bufs=1))
        v = vp.tile([128, mt_u, C], BF16, tag="v")
        # support^T (hi/lo) doubles as the s-matmul rhs (k is folded into
        # the q weights on the host: s = (x G) sup^T with G = wq^T wk), so
        # it stays resident through attention
        stp = ctx.enter_context(tc.tile_pool(name="stp", bufs=1))
        st = stp.tile([128, MW, 2, CT, MS], E4, tag="st")
        o_p = ctx.enter_context(tc.tile_pool(name="op", bufs=1))
        o_h = o_p.tile([128, N // 128, C], E4, tag="o_h")
        o_l = o_p.tile([128, N // 128, C], E4, tag="o_l")
        pwp = ctx.enter_context(tc.tile_pool(name="pwp", bufs=1, side="right"))
        pw_h = pwp.tile([128, 2, CT, 512], E4, tag="pw_h")
        pw_l = pwp.tile([128, 2, CT, 512], E4, tag="pw_l")
        wvp = ctx.enter_context(tc.tile_pool(name="wvp", bufs=1, side="right"))
        wv_h = wvp.tile([128, 2, CT, 512], E4, tag="wv_h")
        wv_l = wvp.tile([128, 2, CT, 512], E4, tag="wv_l")

        maskt = persist.tile([128, mt_u], F32, tag="maskt")
        bias = persist.tile([128, C], BF16, tag="bias")

        # one psum pool shared by the Q and V phases: no bank-reuse
        # barrier at the phase seam
        qkv_ps = ctx.enter_context(ExitStack())
        qps = qkv_ps.enter_context(tc.tile_pool(name="qps", bufs=4, space="PSUM"))

        # ---------------- phase Q: qT[d, n] = (x @ wq^T)^T -----------------
        # wq is dt-major [p, dt, ct, dd]; hi/lo DoubleRow over ct-pairs
        with (
            tc.tile_pool(name="wqp", bufs=1, side="right") as wqp,
            tc.tile_pool(name="xq", bufs=N // NCHUNK) as xqp,
        ):
            wq_h = wqp.tile([128, CT, CT, 128], E4, tag="wq_h")
            wq_l = wqp.tile([128, CT, CT, 128], E4, tag="wq_l")

            def load_xq(i):
                t = xqp.tile([128, 2, CT, NCHUNK], E4, tag="xq")
                nc.sync.dma_start(t[:, :, :, :], xhl[:, i, :, :, :])
                return t

            # ramp-up: x chunk 0 + wq pieces paced with the Q loop, then the
            # remaining x chunks and the K/V working set — all dependency-
            # free, so nothing head-of-line-blocks the DMA issue queue.
            xq0 = xqp.tile([128, 2, CT, NCHUNK], E4, tag="xq")
            nc.sync.dma_start(xq0[:, 0, :, :], xhl[:, 0, 0, :, :])
            nc.sync.dma_start(wq_h[:, 0, :, :], wqh[:, 0, :, :])
            nc.sync.dma_start(wq_l[:, 0, :, :], wql[:, 0, :, :])
            nc.sync.dma_start(xq0[:, 1, :, :], xhl[:, 0, 1, :, :])
            xqs = [xq0]
            for dt in range(1, CT):
                nc.sync.dma_start(wq_h[:, dt, :, :], wqh[:, dt, :, :])
                nc.sync.dma_start(wq_l[:, dt, :, :], wql[:, dt, :, :])
            for i in range(1, NW):
                xqs.append(load_xq(i))
            nc.sync.dma_start(maskt[:], maskf[:])
            nc.sync.dma_start(bias[:], biasb[:])
            for mc in range(MW):
                nc.sync.dma_start(st[:, mc, :, :, :], shl[:, mc, :, :, :])
            for dc in range(2):
                nc.sync.dma_start(wv_h[:, dc, :, :], wvh[:, dc, :, :])
                nc.sync.dma_start(wv_l[:, dc, :, :], wvl[:, dc, :, :])
            for dc in range(2):
                nc.sync.dma_start(pw_h[:, dc, :, :], pwh[:, dc, :, :])
                nc.sync.dma_start(pw_l[:, dc, :, :], pwl[:, dc, :, :])

            for nch in range(NW):
                xq = xqs[nch]
                for dt in range(CT):
                    ps = qps.tile([128, NCHUNK], F32, tag="qps")
                    series = [(wq_h, 0), (wq_h, 1), (wq_l, 0)]
                    for si, (w_, xi) in enumerate(series):
                        for j in range(CT // 2):
                            nc.tensor.matmul(
                                ps[:],
                                lhsT=w_[:, dt, 2 * j:2 * j + 2, :],
                                rhs=xq[:, xi, 2 * j:2 * j + 2, :],
                                start=(si == 0 and j == 0),
                                stop=(si == 2 and j == CT // 2 - 1),
                                perf_mode=DR,
                            )
                    win = slice(nch * NCHUNK, (nch + 1) * NCHUNK)
                    nc.scalar.copy(qt_h[:, dt, win], ps[:])
                    nc.vector.tensor_sub(qt_l[:, dt, win], ps[:], qt_h[:, dt, win])

        # ---------------- phase V: v[m, d] = mask * (support @ wv^T) -------
        for mt in range(mt_u):
            mc, jj = divmod(mt, MS // 128)
            for dc in range(C // 512):
                ps = qps.tile([128, 512], F32, tag="qps")
                series = [(0, wv_h), (0, wv_l), (1, wv_h)]
                for si, (si2, w_) in enumerate(series):
                    for j in range(CT // 2):
                        nc.tensor.matmul(
                            ps[:],
                            lhsT=st[:, mc, si2, 2 * j:2 * j + 2, jj * 128:(jj + 1) * 128],
                            rhs=w_[:, dc, 2 * j:2 * j + 2, :],
                            start=(si == 0 and j == 0),
                            stop=(si == 2 and j == CT // 2 - 1),
                            perf_mode=DR,
                        )
                nc.vector.tensor_scalar_mul(
                    v[:, mt, dc * 512:(dc + 1) * 512],
                    ps[:],
                    maskt[:, mt:mt + 1],
                )

        qkv_ps.close()

        # ---------------- attention: s / exp / transpose / p@v -------------
        with (
            tc.tile_pool(name="sps", bufs=2, space="PSUM") as sps,
            tc.tile_pool(name="ptps", bufs=2, space="PSUM") as ptps,
            tc.tile_pool(name="ops", bufs=2, space="PSUM") as ops,
            tc.tile_pool(name="psb", bufs=3) as psbp,
            tc.tile_pool(name="ptsb", bufs=2) as ptsbp,
            tc.tile_pool(name="obf", bufs=2) as obfp,
            tc.tile_pool(name="stat", bufs=4) as statp,
        ):
            def transpose_and_pv(g, p_sb, o_ps):
                pt_ps = ptps.tile([128, MS], BF16, tag="ptps")
                for j in range(MS // 128):
                    nc.tensor.transpose(
                        pt_ps[:, j * 128:(j + 1) * 128],
                        p_sb[:, j * 128:(j + 1) * 128],
                        ident[:],
                    )
                pt_sb = ptsbp.tile([128, MS], BF16, tag="ptsb")
                nc.vector.tensor_copy(pt_sb[:], pt_ps[:])
                for j in range(MS // 128):
                    mt = g * (MS // 128) + j
                    for dc in range(C // 512):
                        nc.tensor.matmul(
                            o_ps[:, dc * 512:(dc + 1) * 512],
                            lhsT=pt_sb[:, j * 128:(j + 1) * 128],
                            rhs=v[:, mt, dc * 512:(dc + 1) * 512],
                            start=(mt == 0),
                            stop=(mt == mt_u - 1),
                        )

            for ntile in range(N // 128):
                partials = statp.tile([128, 4], F32, tag="partials")
                o_ps = ops.tile([128, C], F32, tag="ops")
                for g in range(M // MS):
                    s_ps = sps.tile([128, MS], F32, tag="sps")
                    series = [(qt_h, 0), (qt_h, 1), (qt_l, 0)]
                    for si, (q_, si2) in enumerate(series):
                        for j in range(CT // 2):
                            nc.tensor.matmul(
                                s_ps[:],
                                lhsT=q_[:, 2 * j:2 * j + 2, ntile * 128:(ntile + 1) * 128],
                                rhs=st[:, g, si2, 2 * j:2 * j + 2, :],
                                start=(si == 0 and j == 0),
                                stop=(si == 2 and j == CT // 2 - 1),
                                perf_mode=DR,
                            )
                    p_sb = psbp.tile([128, MS], BF16, tag="psb")
                    nc.scalar.activation(
                        p_sb[:], s_ps[:], AF.Exp,
                        scale=float(SCALE / WS),
                        accum_out=partials[:, g:g + 1],
                    )
                    if g * MS < mv:
                        transpose_and_pv(g, p_sb, o_ps)
                denom = statp.tile([128, 1], F32, tag="denom")
                nc.vector.reduce_sum(
                    denom[:], partials[:], axis=mybir.AxisListType.X
                )
                recip = statp.tile([128, 1], F32, tag="recip")
                nc.vector.reciprocal(recip[:], denom[:])
                # o_ps = 32*(p@v_true); o_bf = 32*o_true -> fp8 hi/lo
                o_bf = obfp.tile([128, C], BF16, tag="obf")
                nc.vector.tensor_scalar_mul(o_bf[:], o_ps[:], recip[:])
                nc.scalar.copy(o_h[:, ntile, :], o_bf[:])
                nc.vector.tensor_sub(o_l[:, ntile, :], o_bf[:], o_h[:, ntile, :])

        # ---------------- projection with the swapaxes/reshape fold --------
        # out[2t+i, d'] = (sum_c (32 o)[1024 i + c, t] (32 pw)[d', c]) / 1024
        #                 + bias
        with (
            tc.tile_pool(name="fps", bufs=4, space="PSUM") as fps,
            tc.tile_pool(name="fmid", bufs=4) as fmidp,
            tc.tile_pool(name="fsb", bufs=6) as fsbp,
        ):
            out_v = out[:].rearrange("(t two) d -> two t d", two=2)

            def proj_group(i, dc, tt, c0, cw):
                # one [128, cw] psum group of output columns dc*512+c0 ..
                ps = fps.tile([128, cw], F32, tag="fps")
                series = [(o_h, pw_h), (o_h, pw_l), (o_l, pw_h)]
                for si, (o_, w_) in enumerate(series):
                    for j in range(CT // 2):
                        nc.tensor.matmul(
                            ps[:],
                            lhsT=o_[:, CT * i + 2 * j: CT * i + 2 * j + 2, tt * 128:(tt + 1) * 128],
                            rhs=w_[:, dc, 2 * j:2 * j + 2, c0:c0 + cw],
                            start=(si == 0 and j == 0),
                            stop=(si == 2 and j == CT // 2 - 1),
                            perf_mode=DR,
                        )
                f_mid = fmidp.tile([128, cw], F32, tag="fmid")
                nc.scalar.mul(f_mid[:], ps[:], float(1.0 / (WS * WS)))
                f_sb = fsbp.tile([128, cw], F32, tag="fsb")
                nc.vector.tensor_add(
                    f_sb[:], f_mid[:],
                    bias[:, dc * 512 + c0: dc * 512 + c0 + cw],
                )
                nc.sync.dma_start(
                    out_v[i, tt * 128:(tt + 1) * 128, dc * 512 + c0: dc * 512 + c0 + cw],
                    f_sb[:],
                )

            for i in range(2):
                for dc in range(C // 512):
                    for tt in range(CT):
                        if (i, dc, tt) != (1, C // 512 - 1, CT - 1):
                            proj_group(i, dc, tt, 0, 512)
                        else:
                            # split the final group so its drain chain
                            # pipelines instead of sitting in the tail
                            for h in range(2):
                                proj_group(i, dc, tt, h * 256, 256)


def _hilo(a):
    h = np.clip(a, -240.0, 240.0).astype(NPE4)
    l = (a - h.astype(np.float32)).astype(NPE4)
    return np.ascontiguousarray(h), np.ascontiguousarray(l)


def _prep_w_lhs(w):
    # lhsT weights (wk, wq): [p, dt, ct, dd], scaled by WS, fp8 hi/lo
    wt = (w.T * WS).reshape(CT, 128, CT, 128)   # [ct, p, dt, dd]
    return _hilo(wt.transpose(1, 2, 0, 3))


def _prep_w_rhs(w):
    # rhs weights (wv, pw): [p, dc, ct, dd], scaled by WS, fp8 hi/lo
    wt = (w.T * WS).reshape(CT, 128, C // 512, 512)  # [ct, p, dc, dd]
    return _hilo(wt.transpose(1, 2, 0, 3))


def _prep_act(a, win):
    # a [rows, C] -> a.T [C, rows] window-major [p, w, hi/lo, ct, win] fp8
    n = a.shape[0]
    at = a.T.reshape(CT, 128, n // win, win)    # [ct, p, w, win]
    h, l = _hilo(at.transpose(1, 2, 0, 3))      # [p, w, ct, win]
    return np.ascontiguousarray(np.stack([h, l], axis=2))


def _mask_perm(attn_mask):
    # permutation packing unmasked support rows first; tile count for packed v
    mask = np.asarray(attn_mask)
    perm = np.argsort(mask == 0, kind="stable")
    cnt = int((mask != 0).sum())
    mt_u = max(1, min(MT, -(-cnt // 128)))
    return perm, mt_u


def prep_in_maps(x, support, attn_mask, qkv_w, proj_w, proj_b):
    x = np.asarray(x, dtype=np.float32)
    support = np.asarray(support, dtype=np.float32)
    attn_mask = np.asarray(attn_mask)
    qkv_w = np.asarray(qkv_w, dtype=np.float32)
    proj_w = np.asarray(proj_w, dtype=np.float32)
    proj_b = np.asarray(proj_b, dtype=np.float32)

    perm, mt_u = _mask_perm(attn_mask)
    maskp = attn_mask[perm].astype(np.float32)

    # fold k's projection into q's: s = q k^T = x (wq^T wk) sup^T
    G = qkv_w[:C].T @ qkv_w[C:2 * C]
    wq_h, wq_l = _prep_w_lhs(G.T)
    wv_h, wv_l = _prep_w_rhs(qkv_w[2 * C:])
    pw_h, pw_l = _prep_w_rhs(proj_w)
    maskf = np.ascontiguousarray(
        maskp[:mt_u * 128].reshape(mt_u, 128).T
    )
    biasb = np.ascontiguousarray(np.broadcast_to(proj_b, (128, C)).astype(NPBF))

    in_maps = []
    for b in range(B):
        in_maps.append({
            "xhl": _prep_act(x[b], NCHUNK),
            "shl": _prep_act(support[b][perm], MS),
            "wqh": wq_h, "wql": wq_l,
            "wvh": wv_h, "wvl": wv_l,
            "pwh": pw_h, "pwl": pw_l,
            "maskf": maskf,
            "biasb": biasb,
        })
    return in_maps


def kernel(x, support, attn_mask, qkv_w, proj_w, proj_b):
    _, mt_u = _mask_perm(attn_mask)
    if ("nc", mt_u) not in _CACHE:
        _CACHE[("nc", mt_u)] = _build_program(mt_u)
        _CACHE["nc"] = _CACHE[("nc", mt_u)]
    nc = _CACHE[("nc", mt_u)]

    in_maps = prep_in_maps(x, support, attn_mask, qkv_w, proj_w, proj_b)
    res = run_bass_kernel_spmd(nc, in_maps, core_ids=list(range(B)))
    return np.stack([res.results[b]["out"] for b in range(B)], axis=0)


# revision 48
# speedup vs baseline: 1.0033x; 1.0020x over previous
"""Trainium2 Bass kernel for CrossAttention (B=8, N=M=2048, C=1024), fp32 in/out.

Sharding: data-parallel — one batch element per NeuronCore (8 cores).

On top of the bf16 version's optimizations (mask packing, SBUF residency,
Q-first phase order, dependency-clean DMA issue order), the five large
matmuls (q/k/v, s = q k^T, and the output projection) run as fp8-e4m3
DoubleRow matmuls with hi/lo error compensation:

    A @ B  ~=  A_hi @ B_hi + A_hi @ B_lo + A_lo @ B_hi,

where X_hi = fp8(X) and X_lo = fp8(X - X_hi).  DoubleRow processes a
256-deep contraction at 0.5 cycles/row, so three compensated passes cost
0.75x of one bf16 pass per 128-contraction — a 25% tensor-engine saving —
while the residual term makes the operand error ~2x SMALLER than bf16.
All accumulation stays fp32 in PSUM.  Scales: weights are pre-scaled by
32 on the host (w std ~1/32 -> fp8 normal range); activations flow as
32q/32k/32v, the exp scale absorbs 1/1024, o is stored as 32*o, and the
projection descales by 1/1024 on the scalar engine before the bias add.
p (post-exp, unbounded above) and the p@v matmul stay bf16 — p cannot
be ranged into fp8 without max-subtraction.

Per-core computation (batch b):
  qT[d, n] = (x[b] @ wq^T)^T
  kT[d, m] = (support_perm[b] @ wk^T)^T                (all m)
  v[m, d]  = (support_perm[b] @ wv^T) * mask_perm[m]   (m < mv only)
  p = exp(SCALE * q k^T)  (no max subtraction: logits ~ N(0, 8), safe fp32)
  o[n, d] = (p[:, :mv] @ v) / rowsum_all_m(p)
  out[2t+i, d'] = sum_c o[1024 i + c, t] * proj_w[d', c] + proj_b[d']
"""

import sys

sys.path.insert(0, "/opt/trn_rl_repo")

import numpy as np
import ml_dtypes

import concourse.bass as bass
import concourse.tile as tile
from concourse import bacc, mybir
from concourse.bass_utils import run_bass_kernel_spmd
from concourse.masks import make_identity

F32 = mybir.dt.float32
BF16 = mybir.dt.bfloat16
E4 = mybir.dt.float8e4
AF = mybir.ActivationFunctionType
DR = mybir.MatmulPerfMode.DoubleRow
NPBF = ml_dtypes.bfloat16
NPE4 = ml_dtypes.float8_e4m3

B, N, M, C = 8, 2048, 2048, 1024
CT = C // 128          # 8 c-tiles (contraction / channel partition tiles)
MT = M // 128          # 16 m-tiles
SCALE = (C // 8) ** -0.5
WS = 32.0              # host weight upscale into the fp8 normal range
NCHUNK = 256           # q rows computed per chunk in the q phase
MS = 512               # m-chunk for kv build and the s matmul free dim

_CACHE = {}


def _build_program(mt_u):
    nc = bacc.Bacc(
        "TRN2",
        target_bir_lowering=False,
        debug=False,
        enable_asserts=False,
        num_devices=8,
    )

    NW = N // NCHUNK
    MW = M // MS
    xhl = nc.dram_tensor("xhl", [128, NW, 2, CT, NCHUNK], E4, kind="ExternalInput")
    shl = nc.dram_tensor("shl", [128, MW, 2, CT, MS], E4, kind="ExternalInput")
    wqh = nc.dram_tensor("wqh", [128, CT, CT, 128], E4, kind="ExternalInput")
    wql = nc.dram_tensor("wql", [128, CT, CT, 128], E4, kind="ExternalInput")
    wvh = nc.dram_tensor("wvh", [128, 2, CT, 512], E4, kind="ExternalInput")
    wvl = nc.dram_tensor("wvl", [128, 2, CT, 512], E4, kind="ExternalInput")
    pwh = nc.dram_tensor("pwh", [128, 2, CT, 512], E4, kind="ExternalInput")
    pwl = nc.dram_tensor("pwl", [128, 2, CT, 512], E4, kind="ExternalInput")
    maskf = nc.dram_tensor("maskf", [128, mt_u], F32, kind="ExternalInput")
    biasb = nc.dram_tensor("biasb", [128, C], BF16, kind="ExternalInput")
    out = nc.dram_tensor("out", [N, C], F32, kind="ExternalOutput")

    with tile.TileContext(nc, pool_alloc_mode="queue") as tc:
        _trace_kernel(tc, mt_u, xhl, shl, wqh, wql,
                      wvh, wvl, pwh, pwl, maskf, biasb, out)
    nc.compile()
    return nc


def _trace_kernel(tc, mt_u, xhl, shl, wqh, wql,
                  wvh, wvl, pwh, pwl, maskf, biasb, out):
    nc = tc.nc
    mv = mt_u * 128
    NW = N // NCHUNK
    MW = M // MS

    from contextlib import ExitStack

    with ExitStack() as ctx:
        persist = ctx.enter_context(tc.tile_pool(name="persist", bufs=1))

        ident0 = persist.tile([128, 128], F32, tag="ident0")
        make_identity(nc, ident0[:])
        ident = persist.tile([128, 128], BF16, tag="ident")
        nc.scalar.copy(ident[:], ident0[:])

        # long-lived tiles (LIFO pool stack: these close only at the end)
        qtp = ctx.enter_context(tc.tile_pool(name="qtp", bufs=1))
        qt_h = qtp.tile([128, CT, N], E4, tag="qt_h")
        qt_l = qtp.tile([128, CT, N], E4, tag="qt_l")
        vp = ctx.enter_context(tc.tile_pool(name="vp", bufs=1))
        v = vp.tile([128, mt_u, C], BF16, tag="v")
        # support^T (hi/lo) doubles as the s-matmul rhs (k is folded into
        # the q weights on the host: s = (x G) sup^T with G = wq^T wk), so
        # it stays resident through attention
        stp = ctx.enter_context(tc.tile_pool(name="stp", bufs=1))
        st = stp.tile([128, MW, 2, CT, MS], E4, tag="st")
        o_p = ctx.enter_context(tc.tile_pool(name="op", bufs=1))
        o_h = o_p.tile([128, N // 128, C], E4, tag="o_h")
        o_l = o_p.tile([128, N // 128, C], E4, tag="o_l")
        pwp = ctx.enter_context(tc.tile_pool(name="pwp", bufs=1, side="right"))
        pw_h = pwp.tile([128, 2, CT, 512], E4, tag="pw_h")
        pw_l = pwp.tile([128, 2, CT, 512], E4, tag="pw_l")
        wvp = ctx.enter_context(tc.tile_pool(name="wvp", bufs=1, side="right"))
        wv_h = wvp.tile([128, 2, CT, 512], E4, tag="wv_h")
        wv_l = wvp.tile([128, 2, CT, 512], E4, tag="wv_l")

        maskt = persist.tile([128, mt_u], F32, tag="maskt")
        bias = persist.tile([128, C], BF16, tag="bias")

        # one psum pool shared by the Q and V phases: no bank-reuse
        # barrier at the phase seam
        qkv_ps = ctx.enter_context(ExitStack())
        qps = qkv_ps.enter_context(tc.tile_pool(name="qps", bufs=6, space="PSUM"))

        # ---------------- phase Q: qT[d, n] = (x @ wq^T)^T -----------------
        # wq is dt-major [p, dt, ct, dd]; hi/lo DoubleRow over ct-pairs
        with (
            tc.tile_pool(name="wqp", bufs=1, side="right") as wqp,
            tc.tile_pool(name="xq", bufs=N // NCHUNK) as xqp,
        ):
            wq_h = wqp.tile([128, CT, CT, 128], E4, tag="wq_h")
            wq_l = wqp.tile([128, CT, CT, 128], E4, tag="wq_l")

            def load_xq(i):
                t = xqp.tile([128, 2, CT, NCHUNK], E4, tag="xq")
                nc.sync.dma_start(t[:, :, :, :], xhl[:, i, :, :, :])
                return t

            # ramp-up: x chunk 0 + wq pieces paced with the Q loop, then the
            # remaining x chunks and the K/V working set — all dependency-
            # free, so nothing head-of-line-blocks the DMA issue queue.
            xq0 = xqp.tile([128, 2, CT, NCHUNK], E4, tag="xq")
            nc.sync.dma_start(xq0[:, 0, :, :], xhl[:, 0, 0, :, :])
            nc.sync.dma_start(wq_h[:, 0, :, :], wqh[:, 0, :, :])
            nc.sync.dma_start(xq0[:, 1, :, :], xhl[:, 0, 1, :, :])
            nc.sync.dma_start(wq_l[:, 0, :, :], wql[:, 0, :, :])
            xqs = [xq0]
            for dt in range(1, CT):
                nc.sync.dma_start(wq_h[:, dt, :, :], wqh[:, dt, :, :])
                nc.sync.dma_start(wq_l[:, dt, :, :], wql[:, dt, :, :])
            for i in range(1, 3):
                t = xqp.tile([128, 2, CT, NCHUNK], E4, tag="xq")
                nc.sync.dma_start(t[:, 0, :, :], xhl[:, i, 0, :, :])
                nc.sync.dma_start(t[:, 1, :, :], xhl[:, i, 1, :, :])
                xqs.append(t)
            for i in range(3, NW):
                xqs.append(load_xq(i))
            nc.sync.dma_start(maskt[:], maskf[:])
            nc.sync.dma_start(bias[:], biasb[:])
            for mc in range(MW):
                nc.sync.dma_start(st[:, mc, :, :, :], shl[:, mc, :, :, :])
            for dc in range(2):
                nc.sync.dma_start(wv_h[:, dc, :, :], wvh[:, dc, :, :])
                nc.sync.dma_start(wv_l[:, dc, :, :], wvl[:, dc, :, :])
            for dc in range(2):
                nc.sync.dma_start(pw_h[:, dc, :, :], pwh[:, dc, :, :])
                nc.sync.dma_start(pw_l[:, dc, :, :], pwl[:, dc, :, :])

            for nch in range(NW):
                xq = xqs[nch]
                for dt in range(CT):
                    ps = qps.tile([128, NCHUNK], F32, tag="qps")
                    series = [(wq_h, 0), (wq_h, 1), (wq_l, 0)]
                    for si, (w_, xi) in enumerate(series):
                        for j in range(CT // 2):
                            nc.tensor.matmul(
                                ps[:],
                                lhsT=w_[:, dt, 2 * j:2 * j + 2, :],
                                rhs=xq[:, xi, 2 * j:2 * j + 2, :],
                                start=(si == 0 and j == 0),
                                stop=(si == 2 and j == CT // 2 - 1),
                                perf_mode=DR,
                            )
                    win = slice(nch * NCHUNK, (nch + 1) * NCHUNK)
                    nc.scalar.copy(qt_h[:, dt, win], ps[:])
                    nc.vector.tensor_sub(qt_l[:, dt, win], ps[:], qt_h[:, dt, win])

        # ---------------- phase V: v[m, d] = mask * (support @ wv^T) -------
        for mt in range(mt_u):
            mc, jj = divmod(mt, MS // 128)
            for dc in range(C // 512):
                ps = qps.tile([128, 512], F32, tag="qps")
                series = [(0, wv_h), (0, wv_l), (1, wv_h)]
                for si, (si2, w_) in enumerate(series):
                    for j in range(CT // 2):
                        nc.tensor.matmul(
                            ps[:],
                            lhsT=st[:, mc, si2, 2 * j:2 * j + 2, jj * 128:(jj + 1) * 128],
                            rhs=w_[:, dc, 2 * j:2 * j + 2, :],
                            start=(si == 0 and j == 0),
                            stop=(si == 2 and j == CT // 2 - 1),
                            perf_mode=DR,
                        )
                nc.vector.tensor_scalar_mul(
                    v[:, mt, dc * 512:(dc + 1) * 512],
                    ps[:],
                    maskt[:, mt:mt + 1],
                )

        qkv_ps.close()

        # ---------------- attention: s / exp / transpose / p@v -------------
        with (
            tc.tile_pool(name="sps", bufs=2, space="PSUM") as sps,
            tc.tile_pool(name="ptps", bufs=2, space="PSUM") as ptps,
            tc.tile_pool(name="ops", bufs=2, space="PSUM") as ops,
            tc.tile_pool(name="psb", bufs=3) as psbp,
            tc.tile_pool(name="ptsb", bufs=2) as ptsbp,
            tc.tile_pool(name="obf", bufs=2) as obfp,
            tc.tile_pool(name="stat", bufs=4) as statp,
        ):
            def transpose_and_pv(g, p_sb, o_ps):
                pt_ps = ptps.tile([128, MS], BF16, tag="ptps")
                for j in range(MS // 128):
                    nc.tensor.transpose(
                        pt_ps[:, j * 128:(j + 1) * 128],
                        p_sb[:, j * 128:(j + 1) * 128],
                        ident[:],
                    )
                pt_sb = ptsbp.tile([128, MS], BF16, tag="ptsb")
                nc.vector.tensor_copy(pt_sb[:], pt_ps[:])
                for j in range(MS // 128):
                    mt = g * (MS // 128) + j
                    for dc in range(C // 512):
                        nc.tensor.matmul(
                            o_ps[:, dc * 512:(dc + 1) * 512],
                            lhsT=pt_sb[:, j * 128:(j + 1) * 128],
                            rhs=v[:, mt, dc * 512:(dc + 1) * 512],
                            start=(mt == 0),
                            stop=(mt == mt_u - 1),
                        )

            for ntile in range(N // 128):
                partials = statp.tile([128, 4], F32, tag="partials")
                o_ps = ops.tile([128, C], F32, tag="ops")
                for g in range(M // MS):
                    s_ps = sps.tile([128, MS], F32, tag="sps")
                    series = [(qt_h, 0), (qt_h, 1), (qt_l, 0)]
                    for si, (q_, si2) in enumerate(series):
                        for j in range(CT // 2):
                            nc.tensor.matmul(
                                s_ps[:],
                                lhsT=q_[:, 2 * j:2 * j + 2, ntile * 128:(ntile + 1) * 128],
                                rhs=st[:, g, si2, 2 * j:2 * j + 2, :],
                                start=(si == 0 and j == 0),
                                stop=(si == 2 and j == CT // 2 - 1),
                                perf_mode=DR,
                            )
                    p_sb = psbp.tile([128, MS], BF16, tag="psb")
                    nc.scalar.activation(
                        p_sb[:], s_ps[:], AF.Exp,
                        scale=float(SCALE / WS),
                        accum_out=partials[:, g:g + 1],
                    )
                    if g * MS < mv:
                        transpose_and_pv(g, p_sb, o_ps)
                denom = statp.tile([128, 1], F32, tag="denom")
                nc.vector.reduce_sum(
                    denom[:], partials[:], axis=mybir.AxisListType.X
                )
                recip = statp.tile([128, 1], F32, tag="recip")
                nc.vector.reciprocal(recip[:], denom[:])
                # o_ps = 32*(p@v_true); o_bf = 32*o_true -> fp8 hi/lo
                o_bf = obfp.tile([128, C], BF16, tag="obf")
                nc.vector.tensor_scalar_mul(o_bf[:], o_ps[:], recip[:])
                nc.scalar.copy(o_h[:, ntile, :], o_bf[:])
                nc.vector.tensor_sub(o_l[:, ntile, :], o_bf[:], o_h[:, ntile, :])

        # ---------------- projection with the swapaxes/reshape fold --------
        # out[2t+i, d'] = (sum_c (32 o)[1024 i + c, t] (32 pw)[d', c]) / 1024
        #                 + bias
        with (
            tc.tile_pool(name="fps", bufs=4, space="PSUM") as fps,
            tc.tile_pool(name="fmid", bufs=4) as fmidp,
            tc.tile_pool(name="fsb", bufs=6) as fsbp,
        ):
            out_v = out[:].rearrange("(t two) d -> two t d", two=2)

            def proj_group(i, dc, tt, c0, cw):
                # one [128, cw] psum group of output columns dc*512+c0 ..
                ps = fps.tile([128, cw], F32, tag="fps")
                series = [(o_h, pw_h), (o_h, pw_l), (o_l, pw_h)]
                for si, (o_, w_) in enumerate(series):
                    for j in range(CT // 2):
                        nc.tensor.matmul(
                            ps[:],
                            lhsT=o_[:, CT * i + 2 * j: CT * i + 2 * j + 2, tt * 128:(tt + 1) * 128],
                            rhs=w_[:, dc, 2 * j:2 * j + 2, c0:c0 + cw],
                            start=(si == 0 and j == 0),
                            stop=(si == 2 and j == CT // 2 - 1),
                            perf_mode=DR,
                        )
                f_mid = fmidp.tile([128, cw], F32, tag="fmid")
                nc.scalar.mul(f_mid[:], ps[:], float(1.0 / (WS * WS)))
                f_sb = fsbp.tile([128, cw], F32, tag="fsb")
                nc.vector.tensor_add(
                    f_sb[:], f_mid[:],
                    bias[:, dc * 512 + c0: dc * 512 + c0 + cw],
                )
                nc.sync.dma_start(
                    out_v[i, tt * 128:(tt + 1) * 128, dc * 512 + c0: dc * 512 + c0 + cw],
                    f_sb[:],
                )

            for i in range(2):
                for dc in range(C // 512):
                    for tt in range(CT):
                        if (i, dc, tt) != (1, C // 512 - 1, CT - 1):
                            proj_group(i, dc, tt, 0, 512)
                        else:
                            # split the final group so its drain chain
                            # pipelines instead of sitting in the tail
                            for h in range(2):
                                proj_group(i, dc, tt, h * 256, 256)


def _hilo(a):
    h = np.clip(a, -240.0, 240.0).astype(NPE4)
    l = (a - h.astype(np.float32)).astype(NPE4)
    return np.ascontiguousarray(h), np.ascontiguousarray(l)


def _prep_w_lhs(w):
    # lhsT weights (wk, wq): [p, dt, ct, dd], scaled by WS, fp8 hi/lo
    wt = (w.T * WS).reshape(CT, 128, CT, 128)   # [ct, p, dt, dd]
    return _hilo(wt.transpose(1, 2, 0, 3))


def _prep_w_rhs(w):
    # rhs weights (wv, pw): [p, dc, ct, dd], scaled by WS, fp8 hi/lo
    wt = (w.T * WS).reshape(CT, 128, C // 512, 512)  # [ct, p, dc, dd]
    return _hilo(wt.transpose(1, 2, 0, 3))


def _prep_act(a, win):
    # a [rows, C] -> a.T [C, rows] window-major [p, w, hi/lo, ct, win] fp8
    n = a.shape[0]
    at = a.T.reshape(CT, 128, n // win, win)    # [ct, p, w, win]
    h, l = _hilo(at.transpose(1, 2, 0, 3))      # [p, w, ct, win]
    return np.ascontiguousarray(np.stack([h, l], axis=2))


def _mask_perm(attn_mask):
    # permutation packing unmasked support rows first; tile count for packed v
    mask = np.asarray(attn_mask)
    perm = np.argsort(mask == 0, kind="stable")
    cnt = int((mask != 0).sum())
    mt_u = max(1, min(MT, -(-cnt // 128)))
    return perm, mt_u


def prep_in_maps(x, support, attn_mask, qkv_w, proj_w, proj_b):
    x = np.asarray(x, dtype=np.float32)
    support = np.asarray(support, dtype=np.float32)
    attn_mask = np.asarray(attn_mask)
    qkv_w = np.asarray(qkv_w, dtype=np.float32)
    proj_w = np.asarray(proj_w, dtype=np.float32)
    proj_b = np.asarray(proj_b, dtype=np.float32)

    perm, mt_u = _mask_perm(attn_mask)
    maskp = attn_mask[perm].astype(np.float32)

    # fold k's projection into q's: s = q k^T = x (wq^T wk) sup^T
    G = qkv_w[:C].T @ qkv_w[C:2 * C]
    wq_h, wq_l = _prep_w_lhs(G.T)
    wv_h, wv_l = _prep_w_rhs(qkv_w[2 * C:])
    pw_h, pw_l = _prep_w_rhs(proj_w)
    maskf = np.ascontiguousarray(
        maskp[:mt_u * 128].reshape(mt_u, 128).T
    )
    biasb = np.ascontiguousarray(np.broadcast_to(proj_b, (128, C)).astype(NPBF))

    in_maps = []
    for b in range(B):
        in_maps.append({
            "xhl": _prep_act(x[b], NCHUNK),
            "shl": _prep_act(support[b][perm], MS),
            "wqh": wq_h, "wql": wq_l,
            "wvh": wv_h, "wvl": wv_l,
            "pwh": pw_h, "pwl": pw_l,
            "maskf": maskf,
            "biasb": biasb,
        })
    return in_maps


def kernel(x, support, attn_mask, qkv_w, proj_w, proj_b):
    _, mt_u = _mask_perm(attn_mask)
    if ("nc", mt_u) not in _CACHE:
        _CACHE[("nc", mt_u)] = _build_program(mt_u)
        _CACHE["nc"] = _CACHE[("nc", mt_u)]
    nc = _CACHE[("nc", mt_u)]

    in_maps = prep_in_maps(x, support, attn_mask, qkv_w, proj_w, proj_b)
    res = run_bass_kernel_spmd(nc, in_maps, core_ids=list(range(B)))
    return np.stack([res.results[b]["out"] for b in range(B)], axis=0)
